# revision 1
# baseline (speedup 1.0000x reference)
"""Trainium2 Bass kernel for causal self-attention with RoPE (nn_CausalSelfAttention).

Problem (hardcoded): B=2, S=2048, D=1024, H=16 heads, head_dim=64, fp32,
causal mask, RoPE (rotate-half, base 10000), torch-Linear projections
q = x @ Wq.T, kv = x @ Wkv.T interleaved (k even, v odd output channels).

Sharding: 8 cores = 2 batches x 4 head-groups (4 heads each, as 2 row-packed
pairs). Everything per-core is local; no collectives.

Device-side layout choices:
  - All projection activations x are fed transposed (d_in on partitions).
  - q,k are produced TRANSPOSED per head-pair: (128 partitions = 2 heads x 64
    dims, seq free) -- this is directly the scores lhsT/rhs layout.
  - Head dims are permuted on partitions ("paired d-order") so the RoPE
    rotate-half partner is always +16 mod 32 within a 32-partition quadrant,
    implementable with a single DVE stream_shuffle.
  - Scores are computed transposed S^T[k, q] per 128-k-chunk with 2 heads
    row-packed in the 128x128 PE array (contraction=64 each).
  - softmax without max-subtraction (scores ~ N(0,1), |s|<~7 -- safe in fp32);
    exp on ScalarE reads PSUM and writes f32r P^T to SBUF.
  - AV: out^T[d, q] accumulated over k-chunks in PSUM; v carries an extra
    ones-column so row 64 accumulates sum(exp) for free.
  - Normalization + final transpose on host (cheap numpy) from the returned
    (heads, 65, S) tensor.
"""

import numpy as np

B, S, D = 2, 2048, 1024
H, HD = 16, 64
NCORES = 8
ROPE_BASE = 10000.0
NKC = D // 128          # contraction chunks for projections (8)
NSC = S // 128          # seq chunks of 128 (16)
NQB = S // 512          # q blocks of 512 (4)

_CACHE = {}


# --------------------------------------------------------------------------
# host-side index maps
# --------------------------------------------------------------------------
def _dperm():
    """Row r (0..63) -> head-dim d, arranged so the rotate-half partner of the
    dim at row r sits at row (r//32)*32 + (r%32+16)%32 (same quadrant)."""
    p = np.empty(64, np.int64)
    for r in range(64):
        quad, i = divmod(r, 32)
        p[r] = 16 * quad + i if i < 16 else 32 + 16 * quad + (i - 16)
    return p


def _rope_tables():
    inv = 1.0 / (ROPE_BASE ** (np.arange(0, HD, 2, dtype=np.float64) / HD))  # (32,)
    t = np.arange(S, dtype=np.float64)
    fr = t[:, None] * inv[None, :]                    # (S, 32)
    return np.cos(fr), np.sin(fr)                     # float64 (S, 32)


# --------------------------------------------------------------------------
# device kernel builder (same NEFF for all 8 cores)
# --------------------------------------------------------------------------
def _build(reps=1, timing=False):
    key = ("nc", reps, timing)
    if key in _CACHE:
        return _CACHE[key]
    import concourse.tile as tile
    from concourse import bacc, mybir

    f32 = mybir.dt.float32
    f32r = mybir.dt.float32r
    EXP = mybir.ActivationFunctionType.Exp
    MUL = mybir.AluOpType.mult

    nc = bacc.Bacc("TRN2", target_bir_lowering=False, debug=False)
    # timing=True: all real tensors are device-local (Internal) so the PJRT
    # call ships almost nothing through the axon tunnel; wall-clock then
    # approximates RPC + on-chip execution.
    kin = "Internal" if timing else "ExternalInput"
    kout = "Internal" if timing else "ExternalOutput"
    xT = nc.dram_tensor("xT", [NKC, 128, S], f32r, kind=kin).ap()
    wq = nc.dram_tensor("wq", [NKC, 128, 256], f32r, kind=kin).ap()
    wk = nc.dram_tensor("wk", [NKC, 128, 256], f32r, kind=kin).ap()
    wv = nc.dram_tensor("wv", [NKC, 128, 256], f32r, kind=kin).ap()
    cosT = nc.dram_tensor("cosT", [128, S], f32, kind=kin).ap()
    sinT = nc.dram_tensor("sinT", [128, S], f32, kind=kin).ap()
    tri = nc.dram_tensor("tri", [128, 128], f32r, kind=kin).ap()
    vones = nc.dram_tensor("vones", [128, NSC, 4], f32r, kind=kin).ap()
    o = nc.dram_tensor("o", [4, 65, S], f32, kind=kout).ap()
    if timing:
        dummy_in = nc.dram_tensor("dummy_in", [1, 64], f32, kind="ExternalInput").ap()
        dummy_out = nc.dram_tensor("dummy_out", [1, 64], f32, kind="ExternalOutput").ap()

    shuf_mask = [(i + 16) % 32 for i in range(32)]

    with tile.TileContext(nc) as tc:
        with (
            tc.tile_pool(name="cst", bufs=1) as cst,
            tc.tile_pool(name="rope", bufs=3) as rope,
            tc.tile_pool(name="ptp", bufs=6) as ptp,
            tc.tile_pool(name="ost", bufs=3) as ost,
            tc.tile_pool(name="pps", bufs=2, space="PSUM") as pps,
            tc.tile_pool(name="scp", bufs=2, space="PSUM") as scp,
            tc.tile_pool(name="ops", bufs=1, space="PSUM") as ops,
        ):
            xT_sbs = [cst.tile([128, NKC, 512], f32r, tag=f"xT{i}",
                               name=f"xT_sb{i}") for i in range(4)]
            wq_sb = cst.tile([128, NKC, 256], f32r, tag="wq")
            wk_sb = cst.tile([128, NKC, 256], f32r, tag="wk")
            wv_sb = cst.tile([128, NKC, 256], f32r, tag="wv")
            cos_sb = cst.tile([128, S], f32, tag="cos")
            sin_sb = cst.tile([128, S], f32, tag="sin")
            tri_sb = cst.tile([128, 128], f32r, tag="tri")
            qT_sb = cst.tile([128, 2, S], f32r, tag="qT")
            kT_sb = cst.tile([128, 2, S], f32r, tag="kT")
            vx_sb = cst.tile([128, NSC, 4, 65], f32r, tag="vx")

            def proj_qk_sb(dst, w_sb, t, sb, rp):
                """Project one 512-seq block of one head-pair (q or k) + RoPE."""
                ps = pps.tile([128, 512], f32, tag="proj",
                              name=f"ps_{rp}_{id(dst)}_{t}_{sb}")
                for kc in range(NKC):
                    nc.tensor.matmul(
                        ps[:],
                        w_sb[:, kc, t * 128:(t + 1) * 128],
                        xT_sbs[sb][:, kc, :],
                        start=(kc == 0), stop=(kc == NKC - 1))
                sl = slice(sb * 512, (sb + 1) * 512)
                shf = rope.tile([128, 512], f32, tag="shf")
                nc.vector.stream_shuffle(shf[:], ps[:], shuf_mask)
                m2 = rope.tile([128, 512], f32, tag="m2")
                nc.gpsimd.tensor_tensor(m2[:], shf[:], sin_sb[:, sl], MUL)
                m1 = rope.tile([128, 512], f32, tag="m1")
                nc.vector.tensor_tensor(m1[:], ps[:], cos_sb[:, sl], MUL)
                nc.vector.tensor_add(dst[:, t, sl], m1[:], m2[:])

            def proj_v_sc(sc, rp, copy_eng=None):
                psv = pps.tile([128, 256], f32, tag="proj",
                               name=f"psv_{rp}_{sc}")
                for kc in range(NKC):
                    nc.tensor.matmul(
                        psv[:],
                        xT_sbs[sc // 4][:, kc, (sc % 4) * 128:(sc % 4 + 1) * 128],
                        wv_sb[:, kc, :],
                        start=(kc == 0), stop=(kc == NKC - 1))
                # wave-0 copies go to the (still idle) ScalarE so the DVE
                # queue reaches the first mask-multiply sooner
                if copy_eng is None:
                    nc.vector.tensor_copy(
                        vx_sb[:, sc, :, 0:64],
                        psv[:].rearrange("p (h d) -> p h d", h=4))
                else:
                    copy_eng.copy(
                        vx_sb[:, sc, :, 0:64],
                        psv[:].rearrange("p (h d) -> p h d", h=4))

            def attn_qb(pair, qb, rp, pool=None, ptag=None):
                qlo = qb * 512
                pool = pool or ops
                o_ps = [pool.tile([65, 512], f32,
                                  tag=(ptag or f"o{h}"),
                                  name=f"o_ps{rp}_{pair}_{qb}_{h}")
                        for h in range(2)]
                nchunks = 4 * qb + 4

                def emit_sc(c):
                    sc_t = scp.tile([128, 2, 512], f32, tag="sc",
                                    name=f"sc_{rp}_{pair}_{qb}_{c}")
                    for h in range(2):
                        nc.tensor.matmul(
                            sc_t[:, h, :],
                            kT_sb[h * 64:(h + 1) * 64, pair,
                                  c * 128:(c + 1) * 128],
                            qT_sb[h * 64:(h + 1) * 64, pair,
                                  qlo:qlo + 512],
                            start=True, stop=True)
                    return sc_t

                def emit_post(c, sc_t):
                    s = c - 4 * qb        # >=0 on diagonal chunks
                    lo = 0 if s < 0 else 128 * s
                    pt = ptp.tile([128, 2, 512], f32r, tag="pt")
                    nc.scalar.activation(
                        pt[:, :, lo:], sc_t[:, :, lo:], EXP, scale=0.125)
                    if s >= 0:
                        nc.vector.tensor_tensor(
                            pt[:, :, lo:lo + 128],
                            pt[:, :, lo:lo + 128],
                            tri_sb[:].unsqueeze(1).broadcast_to(
                                [128, 2, 128]),
                            MUL)
                    return pt, lo

                def emit_av(c, pt, lo):
                    for h in range(2):
                        nc.tensor.matmul(
                            o_ps[h][:, lo:512],
                            vx_sb[:, c, 2 * pair + h, :],
                            pt[:, h, lo:512],
                            start=(c == 0), stop=(c == nchunks - 1))

                def flush():
                    for h in range(2):
                        o_sb = ost.tile([65, 512], f32, tag="ost")
                        nc.vector.tensor_copy(o_sb[:], o_ps[h][:])
                        nc.sync.dma_start(
                            o[2 * pair + h, :, qlo:qlo + 512], o_sb[:])

                return emit_sc, emit_post, emit_av, flush, nchunks

            # Filler queue: next-wave projection/DMA emission is spliced
            # between attention chunks so the in-order PE/DVE streams
            # interleave it with attention instead of running it as one
            # ScalarE-starving block at each wave boundary.
            fill_q = []

            def fill(n=1):
                for _ in range(min(n, len(fill_q))):
                    fill_q.pop(0)()

            def drain_fill():
                while fill_q:
                    fill_q.pop(0)()

            def attn_qb_run(pair, qb, rp):
                # Software-pipelined emission: scores(c+1) are emitted BEFORE
                # AV(c) so the in-order PE stream never blocks on exp(c) with
                # the next chunk's scores still unissued.
                emit_sc, emit_post, emit_av, flush, n = attn_qb(pair, qb, rp)
                sc_t = emit_sc(0)
                for c in range(n):
                    pt, lo = emit_post(c, sc_t)
                    if c + 1 < n:
                        sc_t = emit_sc(c + 1)
                    emit_av(c, pt, lo)
                flush()
                drain_fill()

            def dma_wave(sb, rp):
                """Load the seq-block-sb slice of x / cos / sin."""
                for kc in range(NKC):
                    nc.sync.dma_start(xT_sbs[sb][:, kc, :],
                                      xT[kc, :, sb * 512:(sb + 1) * 512])
                sl = slice(sb * 512, (sb + 1) * 512)
                nc.sync.dma_start(cos_sb[:, sl], cosT[:, sl])
                nc.sync.dma_start(sin_sb[:, sl], sinT[:, sl])

            if timing:
                dpool = cst.tile([1, 64], f32, tag="dumm", name="dumm")
                nc.sync.dma_start(dpool[:], dummy_in)
                nc.sync.dma_start(dummy_out, dpool[:])
            for rp in range(reps):
                # Wave 0: only what attention q-block 0 needs -- q/k weights +
                # x seq-block 0 -- so ScalarE work starts after ~4MB of DMA,
                # not after the full 11MB input load.
                for kc in range(NKC):
                    nc.sync.dma_start(xT_sbs[0][:, kc, :],
                                      xT[kc, :, 0:512])
                    nc.sync.dma_start(wq_sb[:, kc, :], wq[kc])
                    nc.sync.dma_start(wk_sb[:, kc, :], wk[kc])
                nc.sync.dma_start(cos_sb[:, 0:512], cosT[:, 0:512])
                nc.sync.dma_start(sin_sb[:, 0:512], sinT[:, 0:512])
                nc.sync.dma_start(tri_sb[:], tri)
                proj_qk_sb(qT_sb, wq_sb, 0, 0, rp)
                proj_qk_sb(kT_sb, wk_sb, 0, 0, rp)
                proj_qk_sb(qT_sb, wq_sb, 1, 0, rp)
                proj_qk_sb(kT_sb, wk_sb, 1, 0, rp)
                for kc in range(NKC):
                    nc.sync.dma_start(wv_sb[:, kc, :], wv[kc])
                # ones-columns (index 64 of each head slot); v copies leave them
                nc.sync.dma_start(vx_sb[:, :, :, 64], vones)
                for sc in range(0, 4):
                    proj_v_sc(sc, rp)
                attn_qb_run(0, 0, rp)
                attn_qb_run(1, 0, rp)
                for sb in range(1, 4):
                    dma_wave(sb, rp)
                    proj_qk_sb(qT_sb, wq_sb, 0, sb, rp)
                    proj_qk_sb(kT_sb, wk_sb, 0, sb, rp)
                    proj_qk_sb(qT_sb, wq_sb, 1, sb, rp)
                    proj_qk_sb(kT_sb, wk_sb, 1, sb, rp)
                    for sc in range(4 * sb, 4 * sb + 4):
                        proj_v_sc(sc, rp)
                    attn_qb_run(0, sb, rp)
                    attn_qb_run(1, sb, rp)

    nc.compile()
    _CACHE[key] = nc
    return nc


# --------------------------------------------------------------------------
# host-side sharding / unsharding
# --------------------------------------------------------------------------
def _make_in_maps(x, Wq, Wkv):
    x = np.asarray(x, np.float32)
    Wq = np.asarray(Wq, np.float32)
    Wkv = np.asarray(Wkv, np.float32)

    dp = _dperm()
    cos32, sin32 = _rope_tables()
    sign = np.where((np.arange(128) % 32) < 16, -1.0, 1.0)
    rows64 = np.concatenate([dp, dp])                       # 128 rows, 2 heads
    cosT = cos32[:, rows64 % 32].T.astype(np.float32)       # (128, S)
    sinT = (sin32[:, rows64 % 32].T * sign[:, None]).astype(np.float32)
    tri = (np.arange(128)[:, None] <= np.arange(128)[None, :]).astype(np.float32)

    xT_b = [np.ascontiguousarray(x[b].T).reshape(NKC, 128, S) for b in range(B)]

    in_maps = []
    for c in range(NCORES):
        b, g = divmod(c, 4)
        heads = [4 * g + hh for hh in range(4)]
        qrows = np.concatenate([h * 64 + dp for h in heads])
        krows = np.concatenate([h * 128 + 2 * dp for h in heads])
        vrows = np.concatenate([h * 128 + 2 * np.arange(64) + 1 for h in heads])
        wq_c = np.ascontiguousarray(Wq[qrows, :].T).reshape(NKC, 128, 256)
        wk_c = np.ascontiguousarray(Wkv[krows, :].T).reshape(NKC, 128, 256)
        wv_c = np.ascontiguousarray(Wkv[vrows, :].T).reshape(NKC, 128, 256)
        in_maps.append({
            "xT": xT_b[b], "wq": wq_c, "wk": wk_c, "wv": wv_c,
            "cosT": cosT, "sinT": sinT, "tri": tri,
            "vones": np.ones((128, NSC, 4), np.float32),
        })
    return in_maps


def _assemble(results):
    out = np.empty((B, S, D), np.float32)
    for c in range(NCORES):
        b, g = divmod(c, 4)
        oc = results[c]["o"]                        # (4, 65, S)
        att = oc[:, :64, :] / oc[:, 64:65, :]       # (4, 64, S)
        for hh in range(4):
            head = 4 * g + hh
            out[b, :, head * 64:(head + 1) * 64] = att[hh].T
    return out


def kernel(x, Wq, Wkv, mask=None):
    from concourse.bass_utils import run_bass_kernel_spmd

    nc = _build()
    in_maps = _make_in_maps(x, Wq, Wkv)
    res = run_bass_kernel_spmd(nc, in_maps, core_ids=list(range(NCORES)))
    return _assemble(res.results)



# revision 17
# speedup vs baseline: 1.1512x; 1.1512x over previous
"""Trainium2 Bass kernel for causal self-attention with RoPE (nn_CausalSelfAttention).

Problem (hardcoded): B=2, S=2048, D=1024, H=16 heads, head_dim=64, fp32,
causal mask, RoPE (rotate-half, base 10000), torch-Linear projections
q = x @ Wq.T, kv = x @ Wkv.T interleaved (k even, v odd output channels).

Sharding: 8 cores = 2 batches x 4 head-groups (4 heads each, as 2 row-packed
pairs). Everything per-core is local; no collectives.

Device-side layout choices:
  - All matmul operands are bf16 (PSUM accumulation stays fp32): same PE
    stream rate as f32r at wide tiles, but no 4x penalty below 256-wide
    moving dims, so diagonal score/AV chunks can be causally truncated.
  - All projection activations x are fed transposed (d_in on partitions),
    DRAM-laid-out so each seq-wave loads with ONE descriptor-cheap DMA
    (HWDGE costs ~625ns per dma_start regardless of size -- batch hard).
  - q,k are produced TRANSPOSED per head-pair: (128 partitions = 2 heads x 64
    dims, seq free) -- directly the scores lhsT/rhs layout.
  - Head dims are permuted on partitions ("paired d-order") so the RoPE
    rotate-half partner is always +16 mod 32 within a 32-partition quadrant,
    implementable with a single DVE stream_shuffle.
  - Scores are computed transposed S^T[k, q] per 128-k-chunk with 2 heads
    (contraction=64 each), truncated to the causally live [lo:] columns.
  - softmax without max-subtraction (scores ~ N(0,1), |s|<~7 -- safe);
    exp on ScalarE reads PSUM and writes bf16 P^T to SBUF.
  - AV: out^T[d, q] accumulated over k-chunks in PSUM; v carries an extra
    ones-column so row 64 accumulates sum(exp) for free.
  - Next-wave projection work is spliced between attention chunks through a
    filler queue so the in-order PE stream never idles while ScalarE exps.
  - Normalization + final transpose on host (cheap numpy) from the returned
    (heads, 65, S) tensor.
"""

import numpy as np

B, S, D = 2, 2048, 1024
H, HD = 16, 64
NCORES = 8
ROPE_BASE = 10000.0
NKC = D // 128          # contraction chunks for projections (8)
NSC = S // 128          # seq chunks of 128 (16)
NW = 4                  # seq waves of 512

_CACHE = {}


# --------------------------------------------------------------------------
# host-side index maps
# --------------------------------------------------------------------------
def _dperm():
    """Row r (0..63) -> head-dim d, arranged so the rotate-half partner of the
    dim at row r sits at row (r//32)*32 + (r%32+16)%32 (same quadrant)."""
    p = np.empty(64, np.int64)
    for r in range(64):
        quad, i = divmod(r, 32)
        p[r] = 16 * quad + i if i < 16 else 32 + 16 * quad + (i - 16)
    return p


def _rope_tables():
    inv = 1.0 / (ROPE_BASE ** (np.arange(0, HD, 2, dtype=np.float64) / HD))  # (32,)
    t = np.arange(S, dtype=np.float64)
    fr = t[:, None] * inv[None, :]                    # (S, 32)
    return np.cos(fr), np.sin(fr)                     # float64 (S, 32)


# --------------------------------------------------------------------------
# device kernel builder (same NEFF for all 8 cores)
# --------------------------------------------------------------------------
def _build(reps=1, timing=False):
    key = ("nc", reps, timing)
    if key in _CACHE:
        return _CACHE[key]
    import concourse.tile as tile
    from concourse import bacc, mybir

    f32 = mybir.dt.float32
    bf16 = mybir.dt.bfloat16
    EXP = mybir.ActivationFunctionType.Exp
    MUL = mybir.AluOpType.mult

    nc = bacc.Bacc("TRN2", target_bir_lowering=False, debug=False)
    kin = "Internal" if timing else "ExternalInput"
    kout = "Internal" if timing else "ExternalOutput"
    xT = nc.dram_tensor("xT", [128, NW, NKC, 512], bf16, kind=kin).ap()
    wq = nc.dram_tensor("wq", [128, NKC, 256], bf16, kind=kin).ap()
    wk = nc.dram_tensor("wk", [128, NKC, 256], bf16, kind=kin).ap()
    wv = nc.dram_tensor("wv", [128, NKC, 256], bf16, kind=kin).ap()
    cosT = nc.dram_tensor("cosT", [128, S], bf16, kind=kin).ap()
    sinT = nc.dram_tensor("sinT", [128, S], bf16, kind=kin).ap()
    tri = nc.dram_tensor("tri", [128, 128], bf16, kind=kin).ap()
    o = nc.dram_tensor("o", [4, 65, S], bf16, kind=kout).ap()
    if timing:
        dummy_in = nc.dram_tensor("dummy_in", [1, 64], f32, kind="ExternalInput").ap()
        dummy_out = nc.dram_tensor("dummy_out", [1, 64], f32, kind="ExternalOutput").ap()

    shuf_mask = [(i + 16) % 32 for i in range(32)]

    with tile.TileContext(nc) as tc:
        with (
            tc.tile_pool(name="cst", bufs=1) as cst,
            tc.tile_pool(name="rope", bufs=3) as rope,
            tc.tile_pool(name="ptp", bufs=6) as ptp,
            tc.tile_pool(name="ost", bufs=3) as ost,
            tc.tile_pool(name="pps", bufs=2, space="PSUM") as pps,
            tc.tile_pool(name="scp", bufs=2, space="PSUM") as scp,
            tc.tile_pool(name="ops", bufs=1, space="PSUM") as ops,
        ):
            xT_sbs = [cst.tile([128, NKC, 512], bf16, tag=f"xT{i}",
                               name=f"xT_sb{i}") for i in range(4)]
            wq_sb = cst.tile([128, NKC, 256], bf16, tag="wq")
            wk_sb = cst.tile([128, NKC, 256], bf16, tag="wk")
            wv_sb = cst.tile([128, NKC, 256], bf16, tag="wv")
            cos_sb = cst.tile([128, S], bf16, tag="cos")
            sin_sb = cst.tile([128, S], bf16, tag="sin")
            tri_sb = cst.tile([128, 128], bf16, tag="tri")
            qT_sb = cst.tile([128, 2, S], bf16, tag="qT")
            kT_sb = cst.tile([128, 2, S], bf16, tag="kT")
            vx_sb = cst.tile([128, NSC, 4, 65], bf16, tag="vx")
            warm = cst.tile([128, 16], bf16, tag="warm")

            # ------------------------------------------------------------
            # projection emission units (q/k with RoPE, v with copy)
            # ------------------------------------------------------------
            def qk_units(dst, w_sb, t, sb, rp, which):
                """Units (pe_ns, closure): 4x 2-kc matmul chunks + RoPE tail."""
                ps = pps.tile([128, 512], f32, tag="proj",
                              name=f"ps_{which}_{rp}_{t}_{sb}")

                def mm(kc0):
                    for kc in (kc0, kc0 + 1):
                        nc.tensor.matmul(
                            ps[:],
                            w_sb[:, kc, t * 128:(t + 1) * 128],
                            xT_sbs[sb][:, kc, :],
                            start=(kc == 0), stop=(kc == NKC - 1))

                def rope_tail():
                    # m1 right after the shuffle: both readers of ps done
                    # ASAP, releasing the PSUM buffer for the next group.
                    sl = slice(sb * 512, (sb + 1) * 512)
                    shf = rope.tile([128, 512], f32, tag="shf")
                    nc.vector.stream_shuffle(shf[:], ps[:], shuf_mask)
                    m1 = rope.tile([128, 512], f32, tag="m1")
                    nc.vector.tensor_tensor(m1[:], ps[:], cos_sb[:, sl], MUL)
                    m2 = rope.tile([128, 512], f32, tag="m2")
                    nc.gpsimd.tensor_tensor(m2[:], shf[:], sin_sb[:, sl], MUL)
                    nc.vector.tensor_add(dst[:, t, sl], m1[:], m2[:])

                return [(426, lambda kc0=kc0: mm(kc0))
                        for kc0 in range(0, NKC, 2)] + [(0, rope_tail)]

            def v_units(sc, rp):
                """Units (pe_ns, closure): 2x 4-kc matmul chunks + copy tail."""
                psv = pps.tile([128, 512], f32, tag="proj",
                               name=f"psv_{rp}_{sc}")

                def mm(kc0):
                    for kc in range(kc0, kc0 + 4):
                        nc.tensor.matmul(
                            psv[:, 0:256],
                            xT_sbs[sc // 4][:, kc, (sc % 4) * 128:(sc % 4 + 1) * 128],
                            wv_sb[:, kc, :],
                            start=(kc == 0), stop=(kc == NKC - 1))

                def copy_tail():
                    nc.vector.tensor_copy(
                        vx_sb[:, sc, :, 0:64],
                        psv[:, 0:256].rearrange("p (h d) -> p h d", h=4))

                return [(428, lambda kc0=kc0: mm(kc0)) for kc0 in (0, 4)] \
                    + [(0, copy_tail)]

            # Filler queue: next-wave projection emission is spliced between
            # attention chunks so the in-order PE stream interleaves it with
            # attention instead of idling while ScalarE runs exp.
            fill_q = []

            def fill(budget_ns):
                # Pop units until ~budget_ns of PE work has been spliced in
                # (RoPE/copy tails carry no PE work and ride along free).
                popped = 0
                pops = 0
                while fill_q and popped < budget_ns and pops < 6:
                    pe_ns, fn = fill_q.pop(0)
                    fn()
                    popped += pe_ns
                    pops += 1

            def drain_fill():
                while fill_q:
                    fill_q.pop(0)[1]()

            def run_now(units):
                for _, fn in units:
                    fn()

            def queue(*unit_lists):
                for us in unit_lists:
                    fill_q.extend(us)

            # ------------------------------------------------------------
            # attention
            # ------------------------------------------------------------
            def attn_qb(pair, qb, rp):
                qlo = qb * 512
                o_ps = [ops.tile([65, 512], f32, tag=f"o{h}",
                                 name=f"o_ps{rp}_{pair}_{qb}_{h}")
                        for h in range(2)]
                nchunks = 4 * qb + 4

                def emit_sc(c):
                    s = c - 4 * qb        # >=0 on diagonal chunks
                    lo = 0 if s < 0 else 128 * s
                    sc_t = scp.tile([128, 2, 512], f32, tag="sc",
                                    name=f"sc_{rp}_{pair}_{qb}_{c}")
                    for h in range(2):
                        nc.tensor.matmul(
                            sc_t[:, h, lo:],
                            kT_sb[h * 64:(h + 1) * 64, pair,
                                  c * 128:(c + 1) * 128],
                            qT_sb[h * 64:(h + 1) * 64, pair,
                                  qlo + lo:qlo + 512],
                            start=True, stop=True)
                    return sc_t

                def emit_post(c, sc_t):
                    s = c - 4 * qb
                    lo = 0 if s < 0 else 128 * s
                    pt = ptp.tile([128, 2, 512], bf16, tag="pt")
                    nc.scalar.activation(
                        pt[:, :, lo:], sc_t[:, :, lo:], EXP, scale=0.125)
                    if s >= 0:
                        nc.vector.tensor_tensor(
                            pt[:, :, lo:lo + 128],
                            pt[:, :, lo:lo + 128],
                            tri_sb[:].unsqueeze(1).broadcast_to(
                                [128, 2, 128]),
                            MUL)
                    return pt, lo

                def emit_av(c, pt, lo):
                    for h in range(2):
                        nc.tensor.matmul(
                            o_ps[h][:, lo:512],
                            vx_sb[:, c, 2 * pair + h, :],
                            pt[:, h, lo:512],
                            start=(c == 0), stop=(c == nchunks - 1))

                def flush(final=False):
                    # One SBUF staging tile + ONE output DMA per q-block
                    # (each dma_start costs a full HWDGE slot). The very
                    # last flush splits copies across DVE and ScalarE with
                    # per-head DMAs so the drain chain starts sooner.
                    o_sb = ost.tile([65, 2, 512], bf16, tag="ost")
                    nc.vector.tensor_copy(o_sb[:, 0, :], o_ps[0][:])
                    if final:
                        nc.scalar.copy(o_sb[:, 1, :], o_ps[1][:])
                    else:
                        nc.vector.tensor_copy(o_sb[:, 1, :], o_ps[1][:])
                    nc.sync.dma_start(
                        o[2 * pair:2 * pair + 2, :, qlo:qlo + 512]
                        .rearrange("h p q -> p h q"),
                        o_sb[:])

                return emit_sc, emit_post, emit_av, flush, nchunks

            def attn_qb_run(pair, qb, rp, rate=1, final=False):
                # Software-pipelined emission: scores(c+1) are emitted BEFORE
                # AV(c) so the in-order PE stream never blocks on exp(c) with
                # the next chunk's scores still unissued; filler units keep PE
                # fed while ScalarE works. `rate` = filler units per chunk
                # (int, or per-chunk list) -- ~1 matches the exp-latency gap.
                emit_sc, emit_post, emit_av, flush, n = attn_qb(pair, qb, rp)
                sc_t = emit_sc(0)
                for c in range(n):
                    pt, lo = emit_post(c, sc_t)
                    if c + 1 < n:
                        sc_t = emit_sc(c + 1)
                    emit_av(c, pt, lo)
                    fill(rate[c] if isinstance(rate, (list, tuple)) else rate)
                flush(final=final)

            if timing:
                dpool = cst.tile([1, 64], f32, tag="dumm", name="dumm")
                nc.sync.dma_start(dpool[:], dummy_in)
                nc.sync.dma_start(dummy_out, dpool[:])
            for rp in range(reps):
                # PE p-state warm-up: a tiny matmul on zeroed SBUF so the
                # engine's ramp window elapses during the input DMA.
                wm_ps = pps.tile([128, 512], f32, tag="proj",
                                 name=f"warm_ps{rp}")
                nc.gpsimd.memset(warm[:], 0.0)
                nc.tensor.matmul(wm_ps[0:16, 0:16], warm[:], warm[:],
                                 start=True, stop=True)
                nc.gpsimd.memset(vx_sb[:, :, :, 64:65], 1.0)

                # Batched input DMA, consumption order. Each dma_start costs
                # ~565ns SP-seq + ~625ns HWDGE regardless of size, and all
                # transfers serialize on the DMA engines, so the input set is
                # a handful of transfers sized/ordered by first need.
                nc.sync.dma_start(wq_sb[:, 0:2], wq[:, 0:2])
                nc.sync.dma_start(xT_sbs[0][:, 0:2], xT[:, 0, 0:2])
                nc.sync.dma_start(wk_sb[:, 0:2], wk[:, 0:2])
                nc.sync.dma_start(xT_sbs[0][:, 2:4], xT[:, 0, 2:4])
                nc.sync.dma_start(cos_sb[:, 0:512], cosT[:, 0:512])
                nc.sync.dma_start(sin_sb[:, 0:512], sinT[:, 0:512])
                nc.sync.dma_start(wq_sb[:, 2:8], wq[:, 2:8])
                nc.sync.dma_start(wk_sb[:, 2:8], wk[:, 2:8])
                nc.sync.dma_start(xT_sbs[0][:, 4:6], xT[:, 0, 4:6])
                nc.sync.dma_start(xT_sbs[0][:, 6:8], xT[:, 0, 6:8])
                nc.sync.dma_start(wv_sb[:], wv[:])
                nc.sync.dma_start(tri_sb[:], tri)
                nc.sync.dma_start(xT_sbs[1][:, 0:2], xT[:, 1, 0:2])
                nc.sync.dma_start(xT_sbs[1][:, 2:8], xT[:, 1, 2:8])
                nc.sync.dma_start(cos_sb[:, 512:], cosT[:, 512:])
                nc.sync.dma_start(sin_sb[:, 512:], sinT[:, 512:])
                nc.sync.dma_start(xT_sbs[2][:, 0:2], xT[:, 2, 0:2])
                nc.sync.dma_start(xT_sbs[2][:, 2:8], xT[:, 2, 2:8])
                nc.sync.dma_start(xT_sbs[3][:], xT[:, 3])

                # Wave 0: pair-0 q/k matmuls interleaved per kc-pair (so PE
                # consumes each x0/w piece as it lands), then pair-1 q and all
                # v directly. Later projections are spliced into the attention
                # chunk stream at their LATEST legal position so the filler
                # supply reaches the attention tail.
                uq = qk_units(qT_sb, wq_sb, 0, 0, rp, "q")
                uk = qk_units(kT_sb, wk_sb, 0, 0, rp, "k")
                for i in range(4):
                    uq[i][1]()
                    uk[i][1]()
                uq[4][1]()
                uk[4][1]()
                run_now(qk_units(qT_sb, wq_sb, 1, 0, rp, "q"))
                for sc in range(0, 4):
                    run_now(v_units(sc, rp))
                queue(qk_units(kT_sb, wk_sb, 1, 0, rp, "k"),
                      qk_units(qT_sb, wq_sb, 0, 1, rp, "q"))
                attn_qb_run(0, 0, rp, rate=600)
                drain_fill()
                queue(qk_units(kT_sb, wk_sb, 0, 1, rp, "k"))
                attn_qb_run(1, 0, rp, rate=600)
                drain_fill()
                # (0,1): v1-sc4 first (needed at chunk 4), then next block's q
                # (so its RoPE lands mid-block, clear of boundary congestion)
                queue(v_units(4, rp),
                      qk_units(qT_sb, wq_sb, 1, 1, rp, "q"),
                      v_units(5, rp), v_units(6, rp), v_units(7, rp),
                      qk_units(kT_sb, wk_sb, 1, 1, rp, "k"))
                attn_qb_run(0, 1, rp, rate=[650] * 8 + [450] * 8)
                drain_fill()
                queue(qk_units(qT_sb, wq_sb, 0, 2, rp, "q"),
                      qk_units(kT_sb, wk_sb, 0, 2, rp, "k"))
                attn_qb_run(1, 1, rp, rate=450)
                drain_fill()
                # (0,2): v2 paced to land just before its chunks 8..11
                queue(v_units(8, rp),
                      qk_units(qT_sb, wq_sb, 1, 2, rp, "q"),
                      v_units(9, rp), v_units(10, rp), v_units(11, rp))
                attn_qb_run(0, 2, rp, rate=[650] * 8 + [300] * 16)
                drain_fill()
                queue(qk_units(kT_sb, wk_sb, 1, 2, rp, "k"),
                      qk_units(qT_sb, wq_sb, 0, 3, rp, "q"),
                      qk_units(kT_sb, wk_sb, 0, 3, rp, "k"))
                attn_qb_run(1, 2, rp, rate=[650] * 8 + [300] * 16)
                drain_fill()
                # (0,3): v3 paced to land just before its chunks 12..15
                queue(v_units(12, rp),
                      qk_units(qT_sb, wq_sb, 1, 3, rp, "q"),
                      v_units(13, rp), v_units(14, rp), v_units(15, rp))
                attn_qb_run(0, 3, rp, rate=[650] * 6 + [300] * 10)
                drain_fill()
                queue(qk_units(kT_sb, wk_sb, 1, 3, rp, "k"))
                attn_qb_run(1, 3, rp, rate=450, final=True)

    nc.compile()
    _CACHE[key] = nc
    return nc


# --------------------------------------------------------------------------
# host-side sharding / unsharding
# --------------------------------------------------------------------------
def _make_in_maps(x, Wq, Wkv):
    import ml_dtypes
    BF = ml_dtypes.bfloat16

    x = np.asarray(x, np.float32)
    Wq = np.asarray(Wq, np.float32)
    Wkv = np.asarray(Wkv, np.float32)

    dp = _dperm()
    cos32, sin32 = _rope_tables()
    sign = np.where((np.arange(128) % 32) < 16, -1.0, 1.0)
    rows64 = np.concatenate([dp, dp])                       # 128 rows, 2 heads
    cosT = cos32[:, rows64 % 32].T.astype(BF)               # (128, S)
    sinT = (sin32[:, rows64 % 32].T * sign[:, None]).astype(BF)
    tri = (np.arange(128)[:, None] <= np.arange(128)[None, :]).astype(BF)

    # (128, NW, NKC, 512): partition-major, wave-contiguous per partition
    xT_b = [np.ascontiguousarray(
        x[b].T.reshape(NKC, 128, NW, 512).transpose(1, 2, 0, 3)).astype(BF)
        for b in range(B)]

    in_maps = []
    for c in range(NCORES):
        b, g = divmod(c, 4)
        heads = [4 * g + hh for hh in range(4)]
        qrows = np.concatenate([h * 64 + dp for h in heads])
        krows = np.concatenate([h * 128 + 2 * dp for h in heads])
        vrows = np.concatenate([h * 128 + 2 * np.arange(64) + 1 for h in heads])
        wq_c = np.ascontiguousarray(
            Wq[qrows, :].T.reshape(NKC, 128, 256).transpose(1, 0, 2)).astype(BF)
        wk_c = np.ascontiguousarray(
            Wkv[krows, :].T.reshape(NKC, 128, 256).transpose(1, 0, 2)).astype(BF)
        wv_c = np.ascontiguousarray(
            Wkv[vrows, :].T.reshape(NKC, 128, 256).transpose(1, 0, 2)).astype(BF)
        in_maps.append({
            "xT": xT_b[b], "wq": wq_c, "wk": wk_c, "wv": wv_c,
            "cosT": cosT, "sinT": sinT, "tri": tri,
        })
    return in_maps


def _assemble(results):
    out = np.empty((B, S, D), np.float32)
    for c in range(NCORES):
        b, g = divmod(c, 4)
        oc = np.asarray(results[c]["o"], np.float32)  # (4, 65, S)
        att = oc[:, :64, :] / oc[:, 64:65, :]         # (4, 64, S)
        for hh in range(4):
            head = 4 * g + hh
            out[b, :, head * 64:(head + 1) * 64] = att[hh].T
    return out


def kernel(x, Wq, Wkv, mask=None):
    from concourse.bass_utils import run_bass_kernel_spmd

    nc = _build()
    in_maps = _make_in_maps(x, Wq, Wkv)
    res = run_bass_kernel_spmd(nc, in_maps, core_ids=list(range(NCORES)))
    return _assemble(res.results)
